# revision 21
# baseline (speedup 1.0000x reference)
"""ArcFace softmax loss on 8 TRN2 NeuronCores.

Batch-parallel: 512 rows are split 64 rows/core. Each core streams its
(64, 100000) f32 shard through ScalarE exp (with free-axis accumulate),
fixes up the label column per row (from host-gathered c_y =
costh[i, label_i]), reduces to a partial sum of per-row losses, and an
AllGather + on-device sum produces the mean loss on every core.

Math: logits = SCALE*costh with the label column replaced by
SCALE*cos(acos(c_y)+m). Since SCALE*costh <= 63.4, exp never overflows
f32, so no max-subtraction is needed:
  S_row  = sum_j exp(SCALE*costh[r,j])
  S'_row = S_row - exp(SCALE*c_y) + exp(SCALE*(c_y cos m - sqrt(1-c_y^2) sin m))
  loss   = mean_r( log(S'_row) - SCALE*cos(acos(c_y)+m) )

Hardware constraint that shapes this file: TRN2 engine instructions carry
at most ONE semaphore wait, so the graph is arranged so every instruction
has at most one cross-engine dependency Tile must fence:
  - stream pool has bufs == NT (whole 200KB/partition shard resident in
    SBUF) so no WAR/WAW slot-reuse deps on the streaming Activations;
  - all partition reductions go through PE matmuls against Pool-built
    constants (E pair-collapse matrix, ones); a zero matmul into the
    accumulator "pre-warms" PE's vector clock on the Pool sem so real
    matmuls only wait on their data input;
  - tail DMAs use gpsimd (SWDGE) so they don't pick up HWDGE lane-reuse
    waits on top of their data dependency.
"""

import math

import numpy as np

import concourse.bass as bass
import concourse.bacc as bacc
import concourse.tile as tile
from concourse import mybir
from concourse.bass_utils import run_bass_kernel_spmd

N_CORES = 8
# If True, the 8 per-core partial sums are combined on-device via AllGather
# (+~25us: the tiny collective pays the full ~20us ncfw entry/exit floor).
# If False, each core outputs its partial and the host unshard step sums the
# 8 floats (the batch-dim gather for a loss output).
DEVICE_COMBINE = False
B, C = 512, 100000
RB = B // N_CORES      # 64 rows per core
HALF = C // 2          # 50000: each row is split into 2 partition stripes
# Streaming tile sizes (elems/partition). Front tiles are big (50KB-20KB
# descriptors sustain the ~27GB/s per-engine DMA spec rate); the tail
# shrinks so the final exp after the last byte lands is ~1us, not ~4.5us.
TILES = [5000] * 9 + [2500, 1500, 1000]
assert sum(TILES) == HALF
SCALE = 64.0
MARGIN = 0.5

F32 = mybir.dt.float32
AF = mybir.ActivationFunctionType
ALU = mybir.AluOpType


def _build():
    cos_m = math.cos(MARGIN)
    sin_m = math.sin(MARGIN)

    nc = bacc.Bacc(num_devices=N_CORES)
    costh_ext = nc.declare_dram_parameter("costh", [RB, C], F32, isOutput=False)
    cy_ext = nc.declare_dram_parameter("cy", [RB, 1], F32, isOutput=False)
    out_ext = nc.declare_dram_parameter("out", [1, 1], F32, isOutput=True)

    partial_dram = nc.dram_tensor("partial_dram", [1, 1], F32)
    gath_dram = nc.dram_tensor("gath_dram", [N_CORES, 1], F32, addr_space="Shared")

    # (64,100000) viewed as 128 partition stripes: partition 2r+h = row r,
    # class half h. Keeps every DMA partition-dense (128P) and contiguous.
    x = costh_ext[:, :].rearrange("r (h c) -> (r h) c", h=2)  # (128, 50000)

    with tile.TileContext(nc) as tc:
        with (
            tc.tile_pool(name="stream", bufs=1) as stream,
            tc.tile_pool(name="small", bufs=1) as small,
            tc.tile_pool(name="psum", bufs=1, space="PSUM") as psum_pool,
        ):
            # ---- Pool-engine constants (built while the first DMAs fly)
            ones = small.tile([RB, 1], F32)
            nc.gpsimd.memset(ones[:, :], 1.0)
            negones = small.tile([RB, 1], F32)
            nc.gpsimd.memset(negones[:, :], -1.0)
            zeros = small.tile([128, 1], F32)
            nc.gpsimd.memset(zeros[:, :], 0.0)
            id64 = small.tile([RB, RB], F32)
            nc.gpsimd.memset(id64[:, :], 0.0)
            nc.gpsimd.affine_select(out=id64[:, :], in_=id64[:, :],
                                    compare_op=ALU.not_equal, fill=1.0, base=0,
                                    pattern=[[-1, RB]], channel_multiplier=1)
            emat = small.tile([128, RB], F32)  # E[p,r] = 1 iff p in {2r, 2r+1}
            nc.gpsimd.memset(emat[:, :], 1.0)
            nc.gpsimd.affine_select(out=emat[:, :], in_=emat[:, :],
                                    compare_op=ALU.is_ge, fill=0.0, base=0,
                                    pattern=[[-2, RB]], channel_multiplier=1)
            nc.gpsimd.affine_select(out=emat[:, :], in_=emat[:, :],
                                    compare_op=ALU.is_ge, fill=0.0, base=1,
                                    pattern=[[2, RB]], channel_multiplier=-1)

            # Zero-contribution matmul: initializes the loss accumulator AND
            # (by reading the last-written Pool constant) teaches PE's vector
            # clock about the Pool sem, so later matmuls reading E/ones/id64
            # only need their single data-dependency wait.
            acc_psum = psum_pool.tile([1, 1], F32)
            nc.tensor.matmul(acc_psum[:, :], lhsT=emat[:, 0:1], rhs=zeros[:, :],
                             start=True, stop=False, skip_group_check=True)

            # ---- first two stream tiles issue before anything else on sync
            xts = []
            c0 = 0
            for t, ft in enumerate(TILES):
                if t >= 2:
                    break
                xt = stream.tile([128, ft], F32, tag=f"xt{t}")
                nc.sync.dma_start(out=xt[:, :], in_=x[:, c0:c0 + ft])
                xts.append(xt)
                c0 += ft

            # ---- tiny per-row fixup, depends only on cy
            cy_t = small.tile([RB, 1], F32)
            nc.sync.dma_start(out=cy_t[:, :], in_=cy_ext[:, :])
            sq = small.tile([RB, 1], F32)
            nc.vector.tensor_tensor(out=sq[:, :], in0=cy_t[:, :], in1=cy_t[:, :],
                                    op=ALU.mult)
            om = small.tile([RB, 1], F32)
            nc.vector.tensor_scalar(out=om[:, :], in0=sq[:, :], scalar1=-1.0,
                                    scalar2=1.0, op0=ALU.mult, op1=ALU.add)
            lnom = small.tile([RB, 1], F32)
            nc.scalar.activation(lnom[:, :], om[:, :], AF.Ln)
            rt = small.tile([RB, 1], F32)  # sqrt(om) = exp(0.5*ln(om)):
            nc.scalar.activation(rt[:, :], lnom[:, :], AF.Exp, scale=0.5)
            ca = small.tile([RB, 1], F32)
            nc.vector.tensor_scalar_mul(ca[:, :], cy_t[:, :], cos_m)
            cb = small.tile([RB, 1], F32)
            nc.vector.tensor_scalar_mul(cb[:, :], rt[:, :], sin_m)
            cm = small.tile([RB, 1], F32)
            nc.vector.tensor_tensor(out=cm[:, :], in0=ca[:, :], in1=cb[:, :],
                                    op=ALU.subtract)
            tn = small.tile([RB, 1], F32)  # SCALE * cos(acos(cy)+m)
            nc.vector.tensor_scalar_mul(tn[:, :], cm[:, :], SCALE)
            en = small.tile([RB, 1], F32)
            nc.scalar.activation(en[:, :], tn[:, :], AF.Exp)
            eo = small.tile([RB, 1], F32)
            nc.scalar.activation(eo[:, :], cy_t[:, :], AF.Exp, scale=SCALE)
            delta = small.tile([RB, 1], F32)  # exp(new) - exp(old) per row
            nc.vector.tensor_tensor(out=delta[:, :], in0=en[:, :], in1=eo[:, :],
                                    op=ALU.subtract)
            # fold sum_r(-tn_r) into the loss accumulator now (PSUM accumulate
            # needs no extra sems between matmuls)
            # Ln's spline LUT cannot represent inputs ~1e30, so the log is
            # evaluated on s * 2^-104 (exact power-of-2 scaling in the ACT
            # affine stage); the +104*ln2 compensation rides along in tnshift.
            tnshift = small.tile([RB, 1], F32)
            nc.vector.tensor_scalar(out=tnshift[:, :], in0=tn[:, :], scalar1=1.0,
                                    scalar2=-104.0 * math.log(2.0), op0=ALU.mult,
                                    op1=ALU.add)
            nc.tensor.matmul(acc_psum[:, :], lhsT=tnshift[:, :], rhs=negones[:, :],
                             start=False, stop=False, skip_group_check=True)
            # pre-load s_psum with delta so the E*T matmul lands on top of it
            s_psum = psum_pool.tile([RB, 1], F32)
            nc.tensor.matmul(s_psum[:, :], lhsT=id64[:, :], rhs=delta[:, :],
                             start=True, stop=False, skip_group_check=True)

            # ---- main stream: exp(SCALE*x) with per-partition accumulate
            stats = small.tile([128, len(TILES)], F32)
            for t, ft in enumerate(TILES):
                if t < 2:
                    xt = xts[t]
                else:
                    xt = stream.tile([128, ft], F32, tag=f"xt{t}")
                    nc.sync.dma_start(out=xt[:, :], in_=x[:, c0:c0 + ft])
                    c0 += ft
                nc.scalar.activation(xt[:, :], xt[:, :], AF.Exp, scale=SCALE,
                                     accum_out=stats[:, t:t + 1])

            # ---- per-stripe totals, then pair-collapse to per-row sums
            # (accumulated onto the delta preload: s_psum = delta + E^T . tvec)
            tvec = small.tile([128, 1], F32)
            nc.vector.tensor_reduce(out=tvec[:, :], in_=stats[:, :],
                                    axis=mybir.AxisListType.X, op=ALU.add)
            nc.tensor.matmul(s_psum[:, :], lhsT=emat[:, :], rhs=tvec[:, :],
                             start=False, stop=True, skip_group_check=True)
            lse = small.tile([RB, 1], F32)
            nc.scalar.activation(lse[:, :], s_psum[:, :], AF.Ln, scale=2.0 ** -104)
            nc.tensor.matmul(acc_psum[:, :], lhsT=lse[:, :], rhs=ones[:, :],
                             start=False, stop=True, skip_group_check=True)
            partial_sb = small.tile([1, 1], F32)
            nc.scalar.copy(partial_sb[:, :], acc_psum[:, :])

            if DEVICE_COMBINE:
                # ---- combine the 8 per-core partials on device
                nc.gpsimd.dma_start(out=partial_dram[:, :], in_=partial_sb[:, :])
                nc.gpsimd.collective_compute(
                    "AllGather", ALU.bypass,
                    replica_groups=[list(range(N_CORES))],
                    ins=[partial_dram[:, :]], outs=[gath_dram[:, :]])
                g = small.tile([N_CORES, 1], F32)
                nc.gpsimd.dma_start(out=g[:, :], in_=gath_dram[:, :])
                total_psum = psum_pool.tile([1, 1], F32)
                nc.tensor.matmul(total_psum[:, :], lhsT=g[:, :],
                                 rhs=ones[0:N_CORES, :], start=True, stop=True)
                final = small.tile([1, 1], F32)
                nc.scalar.mul(final[:, :], total_psum[:, :], 1.0 / B)
                nc.gpsimd.dma_start(out=out_ext[:, :], in_=final[:, :])
            else:
                nc.sync.dma_start(out=out_ext[:, :], in_=partial_sb[:, :])

    nc.finalize()  # Bacc.compile(): reg alloc + split multi-sem waits for TRN2
    return nc


_NC = None


def kernel(costh: np.ndarray, label: np.ndarray) -> np.ndarray:
    global _NC
    costh = np.ascontiguousarray(np.asarray(costh, dtype=np.float32))
    label = np.asarray(label).astype(np.int64)
    assert costh.shape == (B, C) and label.shape == (B,)

    cy = costh[np.arange(B), label].astype(np.float32)  # host gather of c_y

    if _NC is None:
        _NC = _build()

    in_maps = []
    for i in range(N_CORES):
        in_maps.append({
            "costh": np.ascontiguousarray(costh[i * RB:(i + 1) * RB]),
            "cy": np.ascontiguousarray(cy[i * RB:(i + 1) * RB].reshape(RB, 1)),
        })

    res = run_bass_kernel_spmd(_NC, in_maps, list(range(N_CORES)))
    if DEVICE_COMBINE:
        out = np.float32(res.results[0]["out"][0, 0])
    else:
        out = np.float32(
            sum(float(res.results[i]["out"][0, 0]) for i in range(N_CORES)) / B)
    kernel.last_exec_time_ns = res.exec_time_ns
    return out


# revision 25
# speedup vs baseline: 1.1775x; 1.1775x over previous
"""ArcFace softmax loss on 8 TRN2 NeuronCores (batch-parallel).

512 rows are split 64 rows/core. Each core streams its (64, 100000) f32
shard through ScalarE exp (with free-axis accumulate) at DMA fabric rate,
fixes up the label column per row (from host-gathered c_y =
costh[i, label_i]), and reduces to a partial sum of its per-row losses.
The host unshard step sums the 8 per-core partials (DEVICE_COMBINE=True
instead does an on-device AllGather + sum, which costs the full ~20us
ncfw collective floor for 4 bytes).

Math: logits = SCALE*costh with the label column replaced by
SCALE*cos(acos(c_y)+m). Since SCALE*costh <= 63.4, exp cannot overflow
f32, so no max-subtraction pass is needed:
  S_row  = sum_j exp(SCALE*costh[r,j])
  S'_row = S_row - exp(SCALE*c_y) + exp(SCALE*(c_y cos m - sqrt(1-c_y^2) sin m))
  loss   = mean_r( log(S'_row) - SCALE*cos(acos(c_y)+m) )

TRN2 specifics that shape the graph:
  - every instruction is arranged to carry at most ONE cross-engine
    dependency (TRN2 engine instructions hold a single semaphore wait;
    Bacc can split more into EVENT_SEMAPHOREs, but those stall the
    sequencers): each streaming tile has its own SBUF slot (the whole
    200KB/partition shard is resident, no WAR/WAW reuse deps), partition
    reductions run as PE matmuls against Pool-built constants (E
    pair-collapse matrix, identity, ones), and a zero matmul reading the
    last Pool constant pre-warms PE's vector clock so real matmuls only
    wait on their data input;
  - per-row sums: exp's accum_out gives per-(row,half) partials in
    stats columns; DVE free-axis reduce -> per-stripe totals; one PE
    matmul with E[p,r]=1 iff p in {2r,2r+1} collapses stripe pairs, on
    top of a PSUM preload of delta (the label-column fixup);
  - Ln's spline LUT cannot represent inputs ~1e30, so the log runs on
    s * 2^-104 (exact power-of-2 scale in the ACT affine stage) and the
    104*ln2 compensation rides in the accumulated -tn term;
  - one manual ACT table load (natural_log_exp_and_others covers ln,
    exp, copy, identity) so no table switches mid-stream or in the tail;
    sqrt(1-c^2) is computed as exp(0.5*ln(1-c^2)) to stay in that set;
  - streaming tile sizes ramp small->big->small: big tiles keep 20KB+
    per-partition DMA descriptors (sustains the ~27GB/s per-engine spec
    rate, ~434GB/s aggregate); the small lead-in starts ACT ~4us
    earlier; the balanced tail keeps ACT tracking the DMA so the
    post-last-byte overhang is receipt latency + one ~1.5us exp.
"""

import math

import numpy as np

import concourse.bacc as bacc
import concourse.tile as tile
from concourse import mybir
from concourse.bass_utils import run_bass_kernel_spmd
from concourse.hw_specs import get_activation_tables

N_CORES = 8
# If True, the 8 per-core partial sums are combined on-device via AllGather
# (+~25us: the tiny collective pays the full ~20us ncfw entry/exit floor).
# If False, each core outputs its partial and the host unshard step sums the
# 8 floats (the batch-dim gather for a loss output).
DEVICE_COMBINE = False
B, C = 512, 100000
RB = B // N_CORES      # 64 rows per core
HALF = C // 2          # 50000: each row is split into 2 partition stripes
# Streaming tile sizes (elems/partition). Front tiles are big (50KB-20KB
# descriptors sustain the ~27GB/s per-engine DMA spec rate); the tail
# shrinks so the final exp after the last byte lands is ~1us, not ~4.5us.
TILES = [1250, 2500] + [5000] * 7 + [4450, 1800, 1800, 1700, 1500]
assert sum(TILES) == HALF
SCALE = 64.0
MARGIN = 0.5

F32 = mybir.dt.float32
AF = mybir.ActivationFunctionType
ALU = mybir.AluOpType


def _build():
    cos_m = math.cos(MARGIN)
    sin_m = math.sin(MARGIN)

    nc = bacc.Bacc(num_devices=N_CORES)
    costh_ext = nc.declare_dram_parameter("costh", [RB, C], F32, isOutput=False)
    cy_ext = nc.declare_dram_parameter("cy", [RB, 1], F32, isOutput=False)
    out_ext = nc.declare_dram_parameter("out", [1, 1], F32, isOutput=True)

    if DEVICE_COMBINE:
        partial_dram = nc.dram_tensor("partial_dram", [1, 1], F32)
        gath_dram = nc.dram_tensor("gath_dram", [N_CORES, 1], F32,
                                   addr_space="Shared")

    # (64,100000) viewed as 128 partition stripes: partition 2r+h = row r,
    # class half h. Keeps every DMA partition-dense (128P) and contiguous.
    x = costh_ext[:, :].rearrange("r (h c) -> (r h) c", h=2)  # (128, 50000)

    with tile.TileContext(nc) as tc:
        with (
            tc.tile_pool(name="stream", bufs=1) as stream,
            tc.tile_pool(name="small", bufs=1) as small,
            tc.tile_pool(name="psum", bufs=1, space="PSUM") as psum_pool,
        ):
            # ---- Pool-engine constants (built while the first DMAs fly)
            ones = small.tile([RB, 1], F32)
            nc.gpsimd.memset(ones[:, :], 1.0)
            negones = small.tile([RB, 1], F32)
            nc.gpsimd.memset(negones[:, :], -1.0)
            zeros = small.tile([128, 1], F32)
            nc.gpsimd.memset(zeros[:, :], 0.0)
            id64 = small.tile([RB, RB], F32)
            nc.gpsimd.memset(id64[:, :], 0.0)
            nc.gpsimd.affine_select(out=id64[:, :], in_=id64[:, :],
                                    compare_op=ALU.not_equal, fill=1.0, base=0,
                                    pattern=[[-1, RB]], channel_multiplier=1)
            emat = small.tile([128, RB], F32)  # E[p,r] = 1 iff p in {2r, 2r+1}
            nc.gpsimd.memset(emat[:, :], 1.0)
            nc.gpsimd.affine_select(out=emat[:, :], in_=emat[:, :],
                                    compare_op=ALU.is_ge, fill=0.0, base=0,
                                    pattern=[[-2, RB]], channel_multiplier=1)
            nc.gpsimd.affine_select(out=emat[:, :], in_=emat[:, :],
                                    compare_op=ALU.is_ge, fill=0.0, base=1,
                                    pattern=[[2, RB]], channel_multiplier=-1)

            # One manual ACT table load: natural_log_exp_and_others holds
            # every function this kernel uses (ln, exp, copy, identity), so
            # Bacc's fixpoint inserts no further loads -- not mid-stream, not
            # in the tail before the final Ln.
            _set_names = list(get_activation_tables(nc.m.arch).keys())
            nc.scalar.add_instruction(mybir.InstLoadActFuncSet(
                name=nc.get_next_instruction_name(),
                act_func_set_id=_set_names.index("natural_log_exp_and_others"),
                ins=[], outs=[]))

            # Zero-contribution matmul: initializes the loss accumulator AND
            # (by reading the last-written Pool constant) teaches PE's vector
            # clock about the Pool sem, so later matmuls reading E/ones/id64
            # only need their single data-dependency wait.
            acc_psum = psum_pool.tile([1, 1], F32)
            nc.tensor.matmul(acc_psum[:, :], lhsT=emat[:, 0:1], rhs=zeros[:, :],
                             start=True, stop=False, skip_group_check=True)

            # ---- tiny per-row fixup, depends only on cy (cy rides the ACT
            # HWDGE queue so the sync sequencer's first issue is tile 0)
            cy_t = small.tile([RB, 1], F32)
            nc.scalar.dma_start(out=cy_t[:, :], in_=cy_ext[:, :])
            sq = small.tile([RB, 1], F32)
            nc.vector.tensor_tensor(out=sq[:, :], in0=cy_t[:, :], in1=cy_t[:, :],
                                    op=ALU.mult)
            om = small.tile([RB, 1], F32)
            nc.vector.tensor_scalar(out=om[:, :], in0=sq[:, :], scalar1=-1.0,
                                    scalar2=1.0, op0=ALU.mult, op1=ALU.add)
            lnom = small.tile([RB, 1], F32)
            nc.scalar.activation(lnom[:, :], om[:, :], AF.Ln)
            rt = small.tile([RB, 1], F32)  # sqrt(om) = exp(0.5*ln(om)):
            nc.scalar.activation(rt[:, :], lnom[:, :], AF.Exp, scale=0.5)
            ca = small.tile([RB, 1], F32)
            nc.vector.tensor_scalar_mul(ca[:, :], cy_t[:, :], cos_m)
            cb = small.tile([RB, 1], F32)
            nc.vector.tensor_scalar_mul(cb[:, :], rt[:, :], sin_m)
            cm = small.tile([RB, 1], F32)
            nc.vector.tensor_tensor(out=cm[:, :], in0=ca[:, :], in1=cb[:, :],
                                    op=ALU.subtract)
            tn = small.tile([RB, 1], F32)  # SCALE * cos(acos(cy)+m)
            nc.vector.tensor_scalar_mul(tn[:, :], cm[:, :], SCALE)
            en = small.tile([RB, 1], F32)
            nc.scalar.activation(en[:, :], tn[:, :], AF.Exp)
            eo = small.tile([RB, 1], F32)
            nc.scalar.activation(eo[:, :], cy_t[:, :], AF.Exp, scale=SCALE)
            delta = small.tile([RB, 1], F32)  # exp(new) - exp(old) per row
            nc.vector.tensor_tensor(out=delta[:, :], in0=en[:, :], in1=eo[:, :],
                                    op=ALU.subtract)
            # fold sum_r(-tn_r) into the loss accumulator now (PSUM accumulate
            # needs no extra sems between matmuls)
            # Ln's spline LUT cannot represent inputs ~1e30, so the log is
            # evaluated on s * 2^-104 (exact power-of-2 scaling in the ACT
            # affine stage); the +104*ln2 compensation rides along in tnshift.
            tnshift = small.tile([RB, 1], F32)
            nc.vector.tensor_scalar(out=tnshift[:, :], in0=tn[:, :], scalar1=1.0,
                                    scalar2=-104.0 * math.log(2.0), op0=ALU.mult,
                                    op1=ALU.add)
            nc.tensor.matmul(acc_psum[:, :], lhsT=tnshift[:, :], rhs=negones[:, :],
                             start=False, stop=False, skip_group_check=True)
            # pre-load s_psum with delta so the E*T matmul lands on top of it
            s_psum = psum_pool.tile([RB, 1], F32)
            nc.tensor.matmul(s_psum[:, :], lhsT=id64[:, :], rhs=delta[:, :],
                             start=True, stop=False, skip_group_check=True)

            # ---- main stream: exp(SCALE*x) with per-partition accumulate
            stats = small.tile([128, len(TILES)], F32)
            c0 = 0
            for t, ft in enumerate(TILES):
                xt = stream.tile([128, ft], F32, tag=f"xt{t}")
                nc.sync.dma_start(out=xt[:, :], in_=x[:, c0:c0 + ft])
                nc.scalar.activation(xt[:, :], xt[:, :], AF.Exp, scale=SCALE,
                                     accum_out=stats[:, t:t + 1])
                c0 += ft

            # ---- per-stripe totals, then pair-collapse to per-row sums
            # (accumulated onto the delta preload: s_psum = delta + E^T . tvec)
            tvec = small.tile([128, 1], F32)
            nc.vector.tensor_reduce(out=tvec[:, :], in_=stats[:, :],
                                    axis=mybir.AxisListType.X, op=ALU.add)
            nc.tensor.matmul(s_psum[:, :], lhsT=emat[:, :], rhs=tvec[:, :],
                             start=False, stop=True, skip_group_check=True)
            lse = small.tile([RB, 1], F32)
            nc.scalar.activation(lse[:, :], s_psum[:, :], AF.Ln, scale=2.0 ** -104)
            nc.tensor.matmul(acc_psum[:, :], lhsT=lse[:, :], rhs=ones[:, :],
                             start=False, stop=True, skip_group_check=True)
            partial_sb = small.tile([1, 1], F32)
            nc.scalar.copy(partial_sb[:, :], acc_psum[:, :])

            if DEVICE_COMBINE:
                # ---- combine the 8 per-core partials on device
                nc.gpsimd.dma_start(out=partial_dram[:, :], in_=partial_sb[:, :])
                nc.gpsimd.collective_compute(
                    "AllGather", ALU.bypass,
                    replica_groups=[list(range(N_CORES))],
                    ins=[partial_dram[:, :]], outs=[gath_dram[:, :]])
                g = small.tile([N_CORES, 1], F32)
                nc.gpsimd.dma_start(out=g[:, :], in_=gath_dram[:, :])
                total_psum = psum_pool.tile([1, 1], F32)
                nc.tensor.matmul(total_psum[:, :], lhsT=g[:, :],
                                 rhs=ones[0:N_CORES, :], start=True, stop=True)
                final = small.tile([1, 1], F32)
                nc.scalar.mul(final[:, :], total_psum[:, :], 1.0 / B)
                nc.gpsimd.dma_start(out=out_ext[:, :], in_=final[:, :])
            else:
                nc.sync.dma_start(out=out_ext[:, :], in_=partial_sb[:, :])

    nc.finalize()  # Bacc.compile(): reg alloc + split multi-sem waits for TRN2
    return nc


_NC = None


def kernel(costh: np.ndarray, label: np.ndarray) -> np.ndarray:
    global _NC
    costh = np.ascontiguousarray(np.asarray(costh, dtype=np.float32))
    label = np.asarray(label).astype(np.int64)
    assert costh.shape == (B, C) and label.shape == (B,)

    cy = costh[np.arange(B), label].astype(np.float32)  # host gather of c_y

    if _NC is None:
        _NC = _build()

    in_maps = []
    for i in range(N_CORES):
        in_maps.append({
            "costh": np.ascontiguousarray(costh[i * RB:(i + 1) * RB]),
            "cy": np.ascontiguousarray(cy[i * RB:(i + 1) * RB].reshape(RB, 1)),
        })

    res = run_bass_kernel_spmd(_NC, in_maps, list(range(N_CORES)))
    if DEVICE_COMBINE:
        out = np.float32(res.results[0]["out"][0, 0])
    else:
        out = np.float32(
            sum(float(res.results[i]["out"][0, 0]) for i in range(N_CORES)) / B)
    kernel.last_exec_time_ns = res.exec_time_ns
    return out

